# revision 9
# baseline (speedup 1.0000x reference)
"""Tensor-parallel MHA prefill kernel for 8 TRN2 NeuronCores.

Sharding: heads across cores (4 Q heads + 1 KV head per core).
Per core: QKV projection (bf16 matmuls, fp32 accum), interleaved RoPE,
causal attention in scores-transposed orientation (softmax denominators
via an appended ones-column in the AV matmul), AllToAll to switch from
head-sharded to sequence-sharded, then the full output projection for
this core's 256 sequence rows. Host only slices/transposes/casts weights
and concatenates the 8 output row-blocks.
"""
import os
import numpy as np
import ml_dtypes

N_CORES = 8
S = 2048          # sequence length
D = 2048          # model dim
NH = 32           # query heads
NKV = 8           # kv heads
HD = 64           # head dim
HPC = NH // N_CORES      # 4 q heads per core
QW = HPC * HD            # 256 q cols per core
SCALE = 1.0 / np.sqrt(HD)

ST = 128          # seq tile
NS = S // ST      # 16 seq tiles
DT = 128          # contraction tile
ND = D // DT      # 16
IC = 512          # i-chunk width for scores
NC_CHUNK = S // IC  # 4
SROWS = S // N_CORES  # 256 output rows per core

_CACHE = {}


def _build():
    import concourse.bass as bass
    from concourse import bacc
    import concourse.mybir as mybir
    from concourse.tile import TileContext
    from concourse.masks import make_identity

    dt = mybir.dt
    nc = bacc.Bacc("TRN2", target_bir_lowering=False, debug=False,
                   num_devices=N_CORES)

    xin = nc.declare_dram_parameter("xin", [S, D], dt.float32, isOutput=False)
    wqkvT = nc.declare_dram_parameter("wqkvT", [D, QW + 2 * HD], dt.bfloat16,
                                      isOutput=False)
    woT = nc.declare_dram_parameter("woT", [D, D], dt.bfloat16, isOutput=False)
    cos5 = nc.declare_dram_parameter("cos5", [S, 160], dt.float32, isOutput=False)
    sin5 = nc.declare_dram_parameter("sin5", [S, 160], dt.float32, isOutput=False)
    out = nc.declare_dram_parameter("out", [SROWS, D], dt.float32, isOutput=True)

    xbf = nc.dram_tensor("xbf", [S, D], dt.bfloat16)
    a2a_in = nc.dram_tensor("a2a_in", [N_CORES, SROWS, SROWS], dt.bfloat16)
    a2a_out = nc.dram_tensor("a2a_out", [N_CORES, SROWS, SROWS], dt.bfloat16)

    with TileContext(nc) as tc:
        const = tc.alloc_tile_pool(name="const", bufs=1)
        ident = const.tile([128, 128], dt.bfloat16, tag="ident")
        make_identity(nc, ident)
        # 4 diagonal-block masks [128, 512]: mask[t][jl, il] = il-jl-128t >= 0
        masks = const.tile([128, 4 * IC], dt.bfloat16, tag="masks")
        nc.gpsimd.memset(masks[:], 1.0)
        for t in range(4):
            nc.gpsimd.affine_select(
                out=masks[:, t * IC:(t + 1) * IC],
                in_=masks[:, t * IC:(t + 1) * IC],
                compare_op=mybir.AluOpType.is_ge,
                fill=0.0, base=-128 * t,
                pattern=[[1, IC]], channel_multiplier=-1,
            )

        # persistent SBUF tensors
        pers = tc.alloc_tile_pool(name="pers", bufs=1)
        wq_sb = [pers.tile([128, QW + 2 * HD], dt.bfloat16, name=f"wq{i}", tag=f"wq{i}")
                 for i in range(ND)]
        qT2 = [pers.tile([128, S], dt.bfloat16, name=f"qT{p}", tag=f"qT{p}") for p in range(2)]
        kT2 = pers.tile([128, S], dt.bfloat16, tag="kT2")
        v_aug = pers.tile([128, NS * (HD + 1)], dt.bfloat16, tag="vaug")
        nc.gpsimd.memset(v_aug[:], 1.0)
        yT = [pers.tile([128, S], dt.bfloat16, name=f"yT{p}", tag=f"yT{p}") for p in range(2)]

        for i in range(ND):
            nc.sync.dma_start(out=wq_sb[i][:], in_=wqkvT[i * DT:(i + 1) * DT, :])

        # ---- phase 1: x cast + transpose (two row-halves) ----
        with tc.tile_pool(name="xt", bufs=1) as xt_pool:
            xT = [xt_pool.tile([128, S], dt.bfloat16, name=f"xT{i}", tag=f"xT{i}")
                  for i in range(ND)]
            H = S // 2
            for h in range(2):
                nc.gpsimd.dma_start(out=xbf[h * H:(h + 1) * H, :],
                                    in_=xin[h * H:(h + 1) * H, :])
                for i in range(ND):
                    nc.sync.dma_start(
                        out=xT[i][:, h * H:(h + 1) * H],
                        in_=xbf[h * H:(h + 1) * H, i * DT:(i + 1) * DT],
                        transpose=True)

            # ---- phase 2: QKV matmul + RoPE + transposes ----
            with (
                tc.tile_pool(name="qkv_ps", bufs=4, space="PSUM") as qkv_ps,
                tc.tile_pool(name="tr_ps", bufs=2, space="PSUM") as tr_ps,
                tc.tile_pool(name="rope", bufs=3) as rope_pool,
                tc.tile_pool(name="qkrot", bufs=1) as qkrot_pool,
            ):
                qkrot = [qkrot_pool.tile([128, QW + HD], dt.bfloat16, name=f"qk{s}", tag=f"qk{s}")
                         for s in range(NS)]
                for s in range(NS):
                    ps = qkv_ps.tile([128, QW + 2 * HD], dt.float32, tag="qkv")
                    for i in range(ND):
                        nc.tensor.matmul(ps[:], xT[i][:, s * ST:(s + 1) * ST],
                                         wq_sb[i][:],
                                         start=(i == 0), stop=(i == ND - 1))
                    cs = rope_pool.tile([128, 160], dt.float32, tag="cos")
                    sn = rope_pool.tile([128, 160], dt.float32, tag="sin")
                    nc.sync.dma_start(out=cs[:], in_=cos5[s * ST:(s + 1) * ST, :])
                    nc.sync.dma_start(out=sn[:], in_=sin5[s * ST:(s + 1) * ST, :])
                    qk_e = ps[:, 0:QW + HD:2]
                    qk_o = ps[:, 1:QW + HD:2]
                    m1 = rope_pool.tile([128, 160], dt.float32, tag="m1")
                    m2 = rope_pool.tile([128, 160], dt.float32, tag="m2")
                    nc.vector.tensor_mul(m1[:], qk_e, cs[:])
                    nc.vector.tensor_mul(m2[:], qk_o, sn[:])
                    nc.vector.tensor_sub(qkrot[s][:, 0:QW + HD:2], m1[:], m2[:])
                    nc.vector.tensor_mul(m1[:], qk_e, sn[:])
                    nc.vector.tensor_mul(m2[:], qk_o, cs[:])
                    nc.vector.tensor_add(qkrot[s][:, 1:QW + HD:2], m1[:], m2[:])
                    # v columns -> v_aug (bf16 cast), ones column preserved
                    nc.scalar.copy(v_aug[:, s * (HD + 1):s * (HD + 1) + HD],
                                   ps[:, QW + HD:QW + 2 * HD])
                    # transposes: q pairs and k
                    for p in range(2):
                        pt = tr_ps.tile([128, 128], dt.bfloat16, tag="tr")
                        nc.tensor.transpose(pt[:], qkrot[s][:, p * 128:(p + 1) * 128],
                                            ident[:])
                        nc.scalar.copy(qT2[p][:, s * ST:(s + 1) * ST], pt[:])
                    pt = tr_ps.tile([128, 128], dt.bfloat16, tag="tr")
                    nc.tensor.transpose(pt[0:HD, :], qkrot[s][:, QW:QW + HD],
                                        ident[:])
                    nc.scalar.copy(kT2[0:HD, s * ST:(s + 1) * ST], pt[0:HD, :])
                # duplicate kT rows to partitions 64-127 for pair packing
                nc.gpsimd.dma_start(out=kT2[HD:128, :], in_=kT2[0:HD, :])

        # ---- preload woT while attention runs ----
        wo_pool = tc.alloc_tile_pool(name="wo_sb", bufs=1)
        wo_sb = [wo_pool.tile([128, D], dt.bfloat16, name=f"wo{m}", tag=f"wo{m}")
                 for m in range(ND)]
        for m in range(ND):
            nc.sync.dma_start(out=wo_sb[m][:], in_=woT[m * DT:(m + 1) * DT, :])

        # ---- phase 3: attention ----
        with (
            tc.tile_pool(name="sc_ps", bufs=4, space="PSUM") as sc_ps,
            tc.tile_pool(name="y_ps", bufs=2, space="PSUM") as y_ps,
            tc.tile_pool(name="yt_ps", bufs=2, space="PSUM") as yt_ps,
            tc.tile_pool(name="expT", bufs=1) as exp_pool,
            tc.tile_pool(name="ytmp", bufs=2) as ytmp_pool,
        ):
            expT = {}
            for p in range(2):   # head pair: heads (2p, 2p+1)
                for c in range(NC_CHUNK):
                    njt = 4 * c + 4
                    for jt in range(njt):
                        for hh in range(2):  # a/b half within pair
                            ps_s = sc_ps.tile([128, IC], dt.float32, tag="sc")
                            nc.tensor.matmul(
                                ps_s[:],
                                kT2[hh * HD:hh * HD + HD,
                                    jt * ST:(jt + 1) * ST],
                                qT2[p][hh * HD:hh * HD + HD,
                                       c * IC:(c + 1) * IC],
                                start=True, stop=True)
                            et = exp_pool.tile([128, IC], dt.bfloat16,
                                               tag=f"e{hh}_{jt}")
                            nc.scalar.activation(et[:], ps_s[:],
                                                 mybir.ActivationFunctionType.Exp,
                                                 scale=float(SCALE))
                            if jt >= 4 * c:
                                toff = jt - 4 * c
                                nc.vector.tensor_mul(
                                    et[:], et[:],
                                    masks[:, toff * IC:(toff + 1) * IC])
                            expT[(hh, jt)] = et
                    for t in range(4):
                        it = 4 * c + t
                        ypair = ytmp_pool.tile([128, 128], dt.bfloat16, tag="yp")
                        for hh in range(2):
                            ps_y = y_ps.tile([128, HD + 1], dt.float32, tag="y")
                            for jt in range(it + 1):
                                nc.tensor.matmul(
                                    ps_y[:],
                                    expT[(hh, jt)][:, t * 128:(t + 1) * 128],
                                    v_aug[:, jt * (HD + 1):(jt + 1) * (HD + 1)],
                                    start=(jt == 0), stop=(jt == it))
                            rec = ytmp_pool.tile([128, 1], dt.float32, tag="rec")
                            nc.vector.reciprocal(rec[:], ps_y[:, HD:HD + 1])
                            nc.vector.tensor_scalar_mul(
                                ypair[:, hh * HD:(hh + 1) * HD],
                                ps_y[:, 0:HD], rec[:])
                        pt = yt_ps.tile([128, 128], dt.bfloat16, tag="yt")
                        nc.tensor.transpose(pt[:], ypair[:], ident[:])
                        nc.scalar.copy(yT[p][:, it * ST:(it + 1) * ST], pt[:])

        # ---- phase 4: A2A + output projection ----
        for p in range(2):
            for j in range(N_CORES):
                nc.sync.dma_start(
                    out=a2a_in[j, p * 128:(p + 1) * 128, :],
                    in_=yT[p][:, j * SROWS:(j + 1) * SROWS])
        nc.gpsimd.collective_compute(
            "AllToAll", mybir.AluOpType.bypass,
            replica_groups=[list(range(N_CORES))],
            ins=[a2a_in[:]], outs=[a2a_out[:]])

        with (
            tc.tile_pool(name="ytf", bufs=1) as ytf_pool,
            tc.tile_pool(name="o_ps", bufs=2, space="PSUM") as o_ps,
            tc.tile_pool(name="o_sb", bufs=2) as o_sb,
        ):
            a2a_flat = a2a_out[:].rearrange("r m s -> (r m) s")
            ytf = [ytf_pool.tile([128, SROWS], dt.bfloat16, name=f"ytf{m}", tag=f"ytf{m}")
                   for m in range(ND)]
            for m in range(ND):
                nc.sync.dma_start(out=ytf[m][:],
                                  in_=a2a_flat[m * DT:(m + 1) * DT, :])
            for st in range(2):
                for nch in range(4):
                    ps_o = o_ps.tile([128, 512], dt.float32, tag="o")
                    for m in range(ND):
                        nc.tensor.matmul(
                            ps_o[:], ytf[m][:, st * 128:(st + 1) * 128],
                            wo_sb[m][:, nch * 512:(nch + 1) * 512],
                            start=(m == 0), stop=(m == ND - 1))
                    ob = o_sb.tile([128, 512], dt.float32, tag="ob")
                    nc.scalar.copy(ob[:], ps_o[:])
                    nc.sync.dma_start(
                        out=out[st * 128:(st + 1) * 128,
                                nch * 512:(nch + 1) * 512],
                        in_=ob[:])

        wo_pool.release()
        pers.release()
        const.release()

    nc.compile()
    return nc


def _numpy_reference(x, freqs_cos, freqs_sin, input_pos, wq, wk, wv, wo,
                     k_cache, v_cache):
    B, S_, _ = x.shape
    n_rep = NH // NKV

    def rope(t, cos, sin):
        tr = t[..., 0::2]
        ti = t[..., 1::2]
        c = cos[None, :, None, :]
        s = sin[None, :, None, :]
        out = np.stack([tr * c - ti * s, tr * s + ti * c], axis=-1)
        return out.reshape(t.shape)

    q = (x @ wq.T).reshape(B, S_, NH, HD)
    k = (x @ wk.T).reshape(B, S_, NKV, HD)
    v = (x @ wv.T).reshape(B, S_, NKV, HD)
    q = rope(q, freqs_cos, freqs_sin).transpose(0, 2, 1, 3)
    k = rope(k, freqs_cos, freqs_sin).transpose(0, 2, 1, 3)
    v = v.transpose(0, 2, 1, 3)
    k_full = np.array(k_cache)
    v_full = np.array(v_cache)
    k_full[:, :, input_pos] = k
    v_full[:, :, input_pos] = v
    mask = np.tril(np.ones((k_full.shape[2], k_full.shape[2]), bool))[input_pos]
    k_rep = np.repeat(k_full, n_rep, axis=1)
    v_rep = np.repeat(v_full, n_rep, axis=1)
    sc = np.einsum("bhsd,bhtd->bhst", q, k_rep) * SCALE
    sc = np.where(mask[None, None], sc, -np.inf)
    sc = sc - sc.max(axis=-1, keepdims=True)
    e = np.exp(sc)
    attn = e / e.sum(axis=-1, keepdims=True)
    y = np.einsum("bhst,bhtd->bhsd", attn, v_rep)
    y = y.transpose(0, 2, 1, 3).reshape(B, S_, NH * HD)
    return (y @ wo.T).astype(np.float32)


def kernel(x, freqs_cos, freqs_sin, input_pos, wq, wk, wv, wo,
           k_cache, v_cache):
    ipos = np.asarray(input_pos)
    if not np.array_equal(ipos, np.arange(S, dtype=ipos.dtype)):
        return _numpy_reference(x, freqs_cos, freqs_sin, ipos, wq, wk, wv, wo,
                                k_cache, v_cache)

    from concourse.bass_utils import run_bass_kernel_spmd

    if "nc" not in _CACHE:
        _CACHE["nc"] = _build()
    nc = _CACHE["nc"]

    bf16 = ml_dtypes.bfloat16
    x2 = np.ascontiguousarray(np.asarray(x, dtype=np.float32)[0])
    cos = np.asarray(freqs_cos, np.float32)
    sin = np.asarray(freqs_sin, np.float32)
    cos5 = np.ascontiguousarray(np.tile(cos, (1, 5)))
    sin5 = np.ascontiguousarray(np.tile(sin, (1, 5)))
    woT = np.ascontiguousarray(np.asarray(wo, np.float32).T.astype(bf16))

    in_maps = []
    for c in range(N_CORES):
        wq_c = np.asarray(wq, np.float32)[c * QW:(c + 1) * QW].T
        wk_c = np.asarray(wk, np.float32)[c * HD:(c + 1) * HD].T
        wv_c = np.asarray(wv, np.float32)[c * HD:(c + 1) * HD].T
        wqkvT = np.ascontiguousarray(
            np.concatenate([wq_c, wk_c, wv_c], axis=1).astype(bf16))
        in_maps.append({
            "xin": x2, "wqkvT": wqkvT, "woT": woT,
            "cos5": cos5, "sin5": sin5,
        })

    res = run_bass_kernel_spmd(nc, in_maps, core_ids=list(range(N_CORES)),
                               trace=bool(os.environ.get("KERNEL_TRACE")))
    _CACHE["last_res"] = res
    rows = [res.results[c]["out"] for c in range(N_CORES)]
    return np.concatenate(rows, axis=0)[None].astype(np.float32)


# revision 10
# speedup vs baseline: 1.2101x; 1.2101x over previous
"""Tensor-parallel MHA prefill kernel for 8 TRN2 NeuronCores.

Sharding: heads across cores (4 Q heads + 1 KV head per core).
Per core: QKV projection (bf16 matmuls, fp32 accum), interleaved RoPE,
causal attention in scores-transposed orientation (softmax denominators
via an appended ones-column in the AV matmul), AllToAll to switch from
head-sharded to sequence-sharded, then the full output projection for
this core's 256 sequence rows. Host only slices/transposes/casts weights
and the input, and concatenates the 8 output row-blocks.
"""
import os
import numpy as np
import ml_dtypes

N_CORES = 8
S = 2048          # sequence length
D = 2048          # model dim
NH = 32           # query heads
HD = 64           # head dim
HPC = NH // N_CORES      # 4 q heads per core
QW = HPC * HD            # 256 q cols per core
QK = QW + HD             # q+k cols (320)
QKV = QW + 2 * HD        # q+k+v cols (384)
SCALE = 1.0 / np.sqrt(HD)

ST = 128          # seq tile
NS = S // ST      # 16 seq tiles
DT = 128          # contraction tile
ND = D // DT      # 16
IC = 512          # i-chunk width for scores
NCH = S // IC     # 4 chunks
SROWS = S // N_CORES  # 256 output rows per core

_CACHE = {}


def _build():
    from concourse import bacc
    import concourse.mybir as mybir
    from concourse.tile import TileContext
    from concourse.masks import make_identity

    dt = mybir.dt
    Exp = mybir.ActivationFunctionType.Exp
    nc = bacc.Bacc("TRN2", target_bir_lowering=False, debug=False,
                   num_devices=N_CORES)

    # host-prepped inputs
    xstr = nc.declare_dram_parameter("xstr", [ND, S, DT], dt.bfloat16,
                                     isOutput=False)  # x in d-stripes
    wqkvT = nc.declare_dram_parameter("wqkvT", [D, QKV], dt.bfloat16,
                                      isOutput=False)
    woT = nc.declare_dram_parameter("woT", [D, D], dt.bfloat16, isOutput=False)
    cos2 = nc.declare_dram_parameter("cos2", [S, QK], dt.float32, isOutput=False)
    sin2 = nc.declare_dram_parameter("sin2", [S, QK], dt.float32, isOutput=False)
    out = nc.declare_dram_parameter("out", [SROWS, D], dt.float32, isOutput=True)

    a2a_in = nc.dram_tensor("a2a_in", [N_CORES, SROWS, SROWS], dt.bfloat16)
    a2a_out = nc.dram_tensor("a2a_out", [N_CORES, SROWS, SROWS], dt.bfloat16)

    with TileContext(nc) as tc:
        const = tc.alloc_tile_pool(name="const", bufs=1)
        ident = const.tile([128, 128], dt.bfloat16, tag="ident")
        make_identity(nc, ident)
        # diagonal triangular mask [128,128]: 1 where jl <= il
        dmask = const.tile([128, 128], dt.bfloat16, tag="dmask")
        nc.gpsimd.memset(dmask[:], 1.0)
        nc.gpsimd.affine_select(
            out=dmask[:], in_=dmask[:], compare_op=mybir.AluOpType.is_ge,
            fill=0.0, base=0, pattern=[[1, 128]], channel_multiplier=-1)

        pers = tc.alloc_tile_pool(name="pers", bufs=1)
        wq_sb = [pers.tile([128, QKV], dt.bfloat16, name=f"wq{i}", tag=f"wq{i}")
                 for i in range(ND)]
        qT2 = [pers.tile([128, S], dt.bfloat16, name=f"qT{p}", tag=f"qT{p}")
               for p in range(2)]
        kT2 = pers.tile([128, S], dt.bfloat16, tag="kT2")
        v_aug = pers.tile([128, NS * (HD + 1)], dt.bfloat16, tag="vaug")
        nc.gpsimd.memset(v_aug[:], 1.0)
        yT = [pers.tile([128, S], dt.bfloat16, name=f"yT{p}", tag=f"yT{p}")
              for p in range(2)]

        # ---- phase 1: x transposes + weight loads ----
        with tc.tile_pool(name="xt", bufs=1) as xt_pool:
            xT = [xt_pool.tile([128, S], dt.bfloat16, name=f"xT{i}", tag=f"xT{i}")
                  for i in range(ND)]
            for i in range(ND):
                nc.sync.dma_start(out=xT[i][:], in_=xstr[i], transpose=True)
                nc.scalar.dma_start(out=wq_sb[i][:],
                                    in_=wqkvT[i * DT:(i + 1) * DT, :])

            # ---- phase 2: QKV matmul + RoPE + transposes ----
            with (
                tc.tile_pool(name="qkv_ps", bufs=5, space="PSUM") as qkv_ps,
                tc.tile_pool(name="tr_ps", bufs=3, space="PSUM") as tr_ps,
                tc.tile_pool(name="rope", bufs=3) as rope_pool,
                tc.tile_pool(name="qkrot", bufs=1) as qkrot_pool,
            ):
                qkrot = [qkrot_pool.tile([128, QK], dt.bfloat16,
                                         name=f"qk{s}", tag=f"qk{s}")
                         for s in range(NS)]
                for s in range(NS):
                    ps = qkv_ps.tile([128, QKV], dt.float32, tag="qkv")
                    for i in range(ND):
                        nc.tensor.matmul(ps[:], xT[i][:, s * ST:(s + 1) * ST],
                                         wq_sb[i][:],
                                         start=(i == 0), stop=(i == ND - 1))
                    cs = rope_pool.tile([128, QK], dt.float32, tag="cos")
                    sn = rope_pool.tile([128, QK], dt.float32, tag="sin")
                    nc.sync.dma_start(out=cs[:], in_=cos2[s * ST:(s + 1) * ST, :])
                    nc.sync.dma_start(out=sn[:], in_=sin2[s * ST:(s + 1) * ST, :])
                    # evict qk straight + pair-swapped (ACT), v (ACT)
                    qk_s = rope_pool.tile([128, QK], dt.float32, tag="qks")
                    qk_w = rope_pool.tile([128, QK], dt.float32, tag="qkw")
                    nc.scalar.copy(qk_s[:], ps[:, 0:QK])
                    nc.scalar.copy(qk_w[:, 0:QK:2], ps[:, 1:QK:2])
                    nc.scalar.copy(qk_w[:, 1:QK:2], ps[:, 0:QK:2])
                    nc.scalar.copy(v_aug[:, s * (HD + 1):s * (HD + 1) + HD],
                                   ps[:, QK:QKV])
                    # rot = qk_s*cos2 + qk_w*sin2pm   (DVE, fp32 2x)
                    t1 = rope_pool.tile([128, QK], dt.float32, tag="t1")
                    t2 = rope_pool.tile([128, QK], dt.float32, tag="t2")
                    nc.vector.tensor_mul(t1[:], qk_s[:], cs[:])
                    nc.vector.tensor_mul(t2[:], qk_w[:], sn[:])
                    nc.vector.tensor_add(qkrot[s][:], t1[:], t2[:])
                    # transposes: q pairs and k
                    for p in range(2):
                        pt = tr_ps.tile([128, 128], dt.bfloat16, tag="tr")
                        nc.tensor.transpose(pt[:],
                                            qkrot[s][:, p * 128:(p + 1) * 128],
                                            ident[:])
                        nc.scalar.copy(qT2[p][:, s * ST:(s + 1) * ST], pt[:])
                    pt = tr_ps.tile([128, 128], dt.bfloat16, tag="tr")
                    nc.tensor.transpose(pt[0:HD, :], qkrot[s][:, QW:QK],
                                        ident[:])
                    nc.scalar.copy(kT2[0:HD, s * ST:(s + 1) * ST], pt[0:HD, :])
                nc.gpsimd.dma_start(out=kT2[HD:128, :], in_=kT2[0:HD, :])

        # ---- preload woT while attention runs ----
        wo_pool = tc.alloc_tile_pool(name="wo_sb", bufs=1)
        wo_sb = [wo_pool.tile([128, D], dt.bfloat16, name=f"wo{m}", tag=f"wo{m}")
                 for m in range(ND)]
        for m in range(ND):
            nc.scalar.dma_start(out=wo_sb[m][:], in_=woT[m * DT:(m + 1) * DT, :])

        # ---- phase 3: attention ----
        with (
            tc.tile_pool(name="sc_ps", bufs=4, space="PSUM") as sc_ps,
            tc.tile_pool(name="y_ps", bufs=2, space="PSUM") as y_ps,
            tc.tile_pool(name="yt_ps", bufs=2, space="PSUM") as yt_ps,
            tc.tile_pool(name="expT", bufs=1) as exp_pool,
            tc.tile_pool(name="ytmp", bufs=2) as ytmp_pool,
        ):
            for p in range(2):   # head pair: heads (2p, 2p+1) of this core
                expT = {}
                for c in range(NCH):
                    njt = 4 * c + 4
                    for jt in range(njt):
                        toff = jt - 4 * c
                        lo = max(toff, 0) * 128  # first causal-valid col
                        w = IC - lo
                        pss = []
                        ets = []
                        for hh in range(2):
                            ps_s = sc_ps.tile([128, IC], dt.float32, tag="sc")
                            nc.tensor.matmul(
                                ps_s[:, 0:w],
                                kT2[hh * HD:hh * HD + HD,
                                    jt * ST:(jt + 1) * ST],
                                qT2[p][hh * HD:hh * HD + HD,
                                       c * IC + lo:(c + 1) * IC],
                                start=True, stop=True)
                            pss.append(ps_s)
                            et = exp_pool.tile([128, IC], dt.bfloat16,
                                               name=f"e{hh}_{jt}",
                                               tag=f"e{hh}_{jt}")
                            ets.append(et)
                        for hh in range(2):
                            nc.scalar.activation(ets[hh][:, lo:IC],
                                                 pss[hh][:, 0:w], Exp,
                                                 scale=float(SCALE))
                            if toff >= 0:
                                nc.vector.tensor_mul(
                                    ets[hh][:, lo:lo + 128],
                                    ets[hh][:, lo:lo + 128], dmask[:])
                            expT[(hh, jt)] = ets[hh]
                    for t in range(4):
                        it = 4 * c + t
                        ypair = ytmp_pool.tile([128, 128], dt.bfloat16, tag="yp")
                        for hh in range(2):
                            ps_y = y_ps.tile([128, HD + 1], dt.float32, tag="y")
                            for jt in range(it + 1):
                                nc.tensor.matmul(
                                    ps_y[:],
                                    expT[(hh, jt)][:, t * 128:(t + 1) * 128],
                                    v_aug[:, jt * (HD + 1):(jt + 1) * (HD + 1)],
                                    start=(jt == 0), stop=(jt == it))
                            rec = ytmp_pool.tile([128, 1], dt.float32, tag="rec")
                            nc.vector.reciprocal(rec[:], ps_y[:, HD:HD + 1])
                            nc.vector.tensor_scalar_mul(
                                ypair[:, hh * HD:(hh + 1) * HD],
                                ps_y[:, 0:HD], rec[:])
                        pt = yt_ps.tile([128, 128], dt.bfloat16, tag="yt")
                        nc.tensor.transpose(pt[:], ypair[:], ident[:])
                        nc.vector.tensor_copy(yT[p][:, it * ST:(it + 1) * ST],
                                              pt[:])

        # ---- phase 4: A2A + output projection ----
        for p in range(2):
            for j in range(N_CORES):
                nc.sync.dma_start(
                    out=a2a_in[j, p * 128:(p + 1) * 128, :],
                    in_=yT[p][:, j * SROWS:(j + 1) * SROWS])
        nc.gpsimd.collective_compute(
            "AllToAll", mybir.AluOpType.bypass,
            replica_groups=[list(range(N_CORES))],
            ins=[a2a_in[:]], outs=[a2a_out[:]])

        with (
            tc.tile_pool(name="ytf", bufs=1) as ytf_pool,
            tc.tile_pool(name="o_ps", bufs=3, space="PSUM") as o_ps,
            tc.tile_pool(name="o_sb", bufs=3) as o_sb,
        ):
            a2a_flat = a2a_out[:].rearrange("r m s -> (r m) s")
            ytf = [ytf_pool.tile([128, SROWS], dt.bfloat16,
                                 name=f"ytf{m}", tag=f"ytf{m}")
                   for m in range(ND)]
            for m in range(ND):
                nc.sync.dma_start(out=ytf[m][:],
                                  in_=a2a_flat[m * DT:(m + 1) * DT, :])
            for st in range(2):
                for nch in range(4):
                    ps_o = o_ps.tile([128, 512], dt.float32, tag="o")
                    for m in range(ND):
                        nc.tensor.matmul(
                            ps_o[:], ytf[m][:, st * 128:(st + 1) * 128],
                            wo_sb[m][:, nch * 512:(nch + 1) * 512],
                            start=(m == 0), stop=(m == ND - 1))
                    ob = o_sb.tile([128, 512], dt.float32, tag="ob")
                    nc.scalar.copy(ob[:], ps_o[:])
                    nc.sync.dma_start(
                        out=out[st * 128:(st + 1) * 128,
                                nch * 512:(nch + 1) * 512],
                        in_=ob[:])

        wo_pool.release()
        pers.release()
        const.release()

    nc.compile()
    return nc


def _numpy_reference(x, freqs_cos, freqs_sin, input_pos, wq, wk, wv, wo,
                     k_cache, v_cache):
    B, S_, _ = x.shape
    NKV = 8
    n_rep = NH // NKV

    def rope(t, cos, sin):
        tr = t[..., 0::2]
        ti = t[..., 1::2]
        c = cos[None, :, None, :]
        s = sin[None, :, None, :]
        o = np.stack([tr * c - ti * s, tr * s + ti * c], axis=-1)
        return o.reshape(t.shape)

    q = (x @ wq.T).reshape(B, S_, NH, HD)
    k = (x @ wk.T).reshape(B, S_, NKV, HD)
    v = (x @ wv.T).reshape(B, S_, NKV, HD)
    q = rope(q, freqs_cos, freqs_sin).transpose(0, 2, 1, 3)
    k = rope(k, freqs_cos, freqs_sin).transpose(0, 2, 1, 3)
    v = v.transpose(0, 2, 1, 3)
    k_full = np.array(k_cache)
    v_full = np.array(v_cache)
    k_full[:, :, input_pos] = k
    v_full[:, :, input_pos] = v
    mask = np.tril(np.ones((k_full.shape[2], k_full.shape[2]), bool))[input_pos]
    k_rep = np.repeat(k_full, n_rep, axis=1)
    v_rep = np.repeat(v_full, n_rep, axis=1)
    sc = np.einsum("bhsd,bhtd->bhst", q, k_rep) * SCALE
    sc = np.where(mask[None, None], sc, -np.inf)
    sc = sc - sc.max(axis=-1, keepdims=True)
    e = np.exp(sc)
    attn = e / e.sum(axis=-1, keepdims=True)
    y = np.einsum("bhst,bhtd->bhsd", attn, v_rep)
    y = y.transpose(0, 2, 1, 3).reshape(B, S_, NH * HD)
    return (y @ wo.T).astype(np.float32)


def _pair_expand(a, sign_odd=False):
    """[S, 32] -> [S, 64]: c -> (c, c) per pair, or (-s, s) if sign_odd."""
    S_, n = a.shape
    o = np.empty((S_, 2 * n), np.float32)
    o[:, 0::2] = -a if sign_odd else a
    o[:, 1::2] = a
    return o


def kernel(x, freqs_cos, freqs_sin, input_pos, wq, wk, wv, wo,
           k_cache, v_cache):
    ipos = np.asarray(input_pos)
    if not np.array_equal(ipos, np.arange(S, dtype=ipos.dtype)):
        return _numpy_reference(np.asarray(x, np.float32),
                                np.asarray(freqs_cos), np.asarray(freqs_sin),
                                ipos, np.asarray(wq), np.asarray(wk),
                                np.asarray(wv), np.asarray(wo),
                                np.asarray(k_cache), np.asarray(v_cache))

    from concourse.bass_utils import run_bass_kernel_spmd

    if "nc" not in _CACHE:
        _CACHE["nc"] = _build()
    nc = _CACHE["nc"]

    bf16 = ml_dtypes.bfloat16
    x2 = np.asarray(x, np.float32)[0].astype(bf16)
    # d-stripes [16, 2048, 128], each contiguous
    xstr = np.ascontiguousarray(x2.reshape(S, ND, DT).transpose(1, 0, 2))
    cos = np.asarray(freqs_cos, np.float32)
    sin = np.asarray(freqs_sin, np.float32)
    c2 = _pair_expand(cos)                  # [S, 64] (c, c)
    s2 = _pair_expand(sin, sign_odd=True)   # [S, 64] (-s, +s)
    cos2 = np.ascontiguousarray(np.tile(c2, (1, 5)))   # [S, 320]
    sin2 = np.ascontiguousarray(np.tile(s2, (1, 5)))
    woT = np.ascontiguousarray(np.asarray(wo, np.float32).T.astype(bf16))

    in_maps = []
    for c in range(N_CORES):
        wq_c = np.asarray(wq, np.float32)[c * QW:(c + 1) * QW].T
        wk_c = np.asarray(wk, np.float32)[c * HD:(c + 1) * HD].T
        wv_c = np.asarray(wv, np.float32)[c * HD:(c + 1) * HD].T
        wqkvT = np.ascontiguousarray(
            np.concatenate([wq_c, wk_c, wv_c], axis=1).astype(bf16))
        in_maps.append({
            "xstr": xstr, "wqkvT": wqkvT, "woT": woT,
            "cos2": cos2, "sin2": sin2,
        })

    res = run_bass_kernel_spmd(nc, in_maps, core_ids=list(range(N_CORES)),
                               trace=bool(os.environ.get("KERNEL_TRACE")))
    _CACHE["last_res"] = res
    rows = [res.results[c]["out"] for c in range(N_CORES)]
    return np.concatenate(rows, axis=0)[None].astype(np.float32)


# revision 14
# speedup vs baseline: 1.2887x; 1.0649x over previous
"""Tensor-parallel MHA prefill kernel for 8 TRN2 NeuronCores.

Sharding: heads across cores (4 Q heads + 1 KV head per core).
Per core: QKV projection (bf16 matmuls, fp32 accum), interleaved RoPE,
causal attention in scores-transposed orientation (softmax denominators
via an appended ones-column in the AV matmul, accumulated over 4-j-tile
subgroups into SBUF), pair-split AllToAll overlapped with the split
output projection for this core's 256 sequence rows. Host only
slices/transposes/casts weights and the input, and concatenates the 8
output row-blocks.
"""
import os
import numpy as np
import ml_dtypes

N_CORES = 8
S = 2048
D = 2048
NH = 32
HD = 64
HPC = NH // N_CORES      # 4 q heads per core
QW = HPC * HD            # 256
QK = QW + HD             # 320
QKV = QW + 2 * HD        # 384
SCALE = 1.0 / np.sqrt(HD)

ST = 128
NS = S // ST             # 16
DT = 128
ND = D // DT             # 16
NXB = 8                  # stripes transposed via xbar DMA (rest via PE)
IC = 512
NCH = S // IC            # 4
SROWS = S // N_CORES     # 256
HD1 = HD + 1             # 65

_CACHE = {}


def _build():
    from concourse import bacc
    import concourse.mybir as mybir
    from concourse.tile import TileContext
    from concourse.masks import make_identity

    dt = mybir.dt
    Exp = mybir.ActivationFunctionType.Exp
    nc = bacc.Bacc("TRN2", target_bir_lowering=False, debug=False,
                   num_devices=N_CORES)

    xstr = nc.declare_dram_parameter("xstr", [ND, S, DT], dt.bfloat16,
                                     isOutput=False)
    wqkvT = nc.declare_dram_parameter("wqkvT", [128, ND * QKV], dt.bfloat16,
                                      isOutput=False)
    woT = nc.declare_dram_parameter("woT", [128, ND * D], dt.bfloat16,
                                    isOutput=False)
    cos2 = nc.declare_dram_parameter("cos2", [128, NS * QK], dt.float32,
                                     isOutput=False)
    sin2 = nc.declare_dram_parameter("sin2", [128, NS * QK], dt.float32,
                                     isOutput=False)
    out = nc.declare_dram_parameter("out", [SROWS, D], dt.float32, isOutput=True)

    a2a_in = [nc.dram_tensor(f"a2a_in{p}", [N_CORES, 128, SROWS], dt.bfloat16)
              for p in range(2)]
    a2a_out = [nc.dram_tensor(f"a2a_out{p}", [N_CORES, 128, SROWS], dt.bfloat16)
               for p in range(2)]

    with TileContext(nc) as tc:
        const = tc.alloc_tile_pool(name="const", bufs=1)
        ident = const.tile([128, 128], dt.bfloat16, tag="ident")
        make_identity(nc, ident)
        dmask = const.tile([128, 128], dt.bfloat16, tag="dmask")
        nc.gpsimd.memset(dmask[:], 1.0)
        nc.gpsimd.affine_select(
            out=dmask[:], in_=dmask[:], compare_op=mybir.AluOpType.is_ge,
            fill=0.0, base=0, pattern=[[1, 128]], channel_multiplier=-1)

        pers = tc.alloc_tile_pool(name="pers", bufs=1)
        wq_sb = pers.tile([128, ND * QKV], dt.bfloat16, tag="wq")
        cs_sb = pers.tile([128, NS * QK], dt.float32, tag="cs")
        sn_sb = pers.tile([128, NS * QK], dt.float32, tag="sn")
        qT2 = [pers.tile([128, S], dt.bfloat16, name=f"qT{p}", tag=f"qT{p}")
               for p in range(2)]
        kT2 = pers.tile([128, S], dt.bfloat16, tag="kT2")
        v_aug = pers.tile([128, NS * HD1], dt.bfloat16, tag="vaug")
        nc.gpsimd.memset(v_aug[:], 1.0)

        # ---- phase 1+2: x transposes (xbar on sync for stripes < NXB,
        #      PE for the rest), then QKV + RoPE + q/k transposes ----
        with (
            tc.tile_pool(name="xt", bufs=1) as xt_pool,
            tc.tile_pool(name="xn", bufs=4) as xn_pool,
        ):
            xT = [xt_pool.tile([128, S], dt.bfloat16, name=f"xT{i}",
                               tag=f"xT{i}") for i in range(ND)]
            nc.gpsimd.dma_start(out=wq_sb[:], in_=wqkvT[:])
            for i in range(NXB):
                nc.sync.dma_start(out=xT[i][:], in_=xstr[i], transpose=True)
            nc.gpsimd.dma_start(out=cs_sb[:], in_=cos2[:])
            nc.gpsimd.dma_start(out=sn_sb[:], in_=sin2[:])

            with (
                tc.tile_pool(name="qkv_ps", bufs=5, space="PSUM") as qkv_ps,
                tc.tile_pool(name="tr_ps", bufs=3, space="PSUM") as tr_ps,
                tc.tile_pool(name="rope", bufs=3) as rope_pool,
                tc.tile_pool(name="qkrot", bufs=1) as qkrot_pool,
            ):
                # PE transposes for stripes NXB..15, fed by native x rows
                for s in range(NS):
                    xn = xn_pool.tile([128, (ND - NXB) * DT], dt.bfloat16,
                                      tag="xn")
                    nc.scalar.dma_start(
                        out=xn[:],
                        in_=xstr[NXB:ND, s * ST:(s + 1) * ST, :]
                        .rearrange("i s d -> s i d"))
                    for i in range(NXB, ND):
                        pt = tr_ps.tile([128, 128], dt.bfloat16, tag="tr")
                        nc.tensor.transpose(
                            pt[:], xn[:, (i - NXB) * DT:(i - NXB + 1) * DT],
                            ident[:])
                        nc.vector.tensor_copy(xT[i][:, s * ST:(s + 1) * ST],
                                              pt[:])
                qkrot = [qkrot_pool.tile([128, QK], dt.bfloat16,
                                         name=f"qk{s}", tag=f"qk{s}")
                         for s in range(NS)]
                for s in range(NS):
                    ps = qkv_ps.tile([128, QKV], dt.float32, tag="qkv")
                    for i in range(ND):
                        nc.tensor.matmul(ps[:], xT[i][:, s * ST:(s + 1) * ST],
                                         wq_sb[:, i * QKV:(i + 1) * QKV],
                                         start=(i == 0), stop=(i == ND - 1))
                    qk_w = rope_pool.tile([128, QK], dt.float32, tag="qkw")
                    nc.scalar.copy(qk_w[:, 0:QK:2], ps[:, 1:QK:2])
                    nc.scalar.copy(qk_w[:, 1:QK:2], ps[:, 0:QK:2])
                    nc.scalar.copy(v_aug[:, s * HD1:s * HD1 + HD],
                                   ps[:, QK:QKV])
                    t1 = rope_pool.tile([128, QK], dt.float32, tag="t1")
                    t2 = rope_pool.tile([128, QK], dt.float32, tag="t2")
                    nc.vector.tensor_mul(t1[:], ps[:, 0:QK],
                                         cs_sb[:, s * QK:(s + 1) * QK])
                    nc.vector.tensor_mul(t2[:], qk_w[:],
                                         sn_sb[:, s * QK:(s + 1) * QK])
                    nc.vector.tensor_add(qkrot[s][:], t1[:], t2[:])
                    for p in range(2):
                        pt = tr_ps.tile([128, 128], dt.bfloat16, tag="tr")
                        nc.tensor.transpose(
                            pt[:], qkrot[s][:, p * 128:(p + 1) * 128], ident[:])
                        nc.scalar.copy(qT2[p][:, s * ST:(s + 1) * ST], pt[:])
                    pt = tr_ps.tile([128, 128], dt.bfloat16, tag="tr")
                    nc.tensor.transpose(pt[0:HD, :], qkrot[s][:, QW:QK],
                                        ident[:])
                    nc.scalar.copy(kT2[0:HD, s * ST:(s + 1) * ST], pt[0:HD, :])
                    nc.scalar.copy(kT2[HD:128, s * ST:(s + 1) * ST],
                                   pt[0:HD, :])

        # ---- preload woT (one 8MB DMA) while attention runs ----
        wo_pool = tc.alloc_tile_pool(name="wo_sb", bufs=1)
        wo_sb = wo_pool.tile([128, ND * D], dt.bfloat16, tag="wo")
        nc.gpsimd.dma_start(out=wo_sb[:], in_=woT[:])

        # ---- phase 3: attention (pair-major; AV in 4-jt subgroups) ----
        with (
            tc.tile_pool(name="sc_ps", bufs=3, space="PSUM") as sc_ps,
            tc.tile_pool(name="av_ps", bufs=1, space="PSUM") as av_ps,
            tc.tile_pool(name="yt_ps", bufs=1, space="PSUM") as yt_ps,
            tc.tile_pool(name="expT", bufs=2) as exp_pool,
            tc.tile_pool(name="acc", bufs=1) as acc_pool,
            tc.tile_pool(name="ytmp", bufs=2) as ytmp_pool,
            tc.tile_pool(name="ystage", bufs=2) as ystage_pool,
        ):
            for p in range(2):
                accs = {}
                for hh in range(2):
                    accs[hh] = acc_pool.tile([128, 4 * HD1], dt.float32,
                                             name=f"acc{hh}", tag=f"acc{hh}")
                for c in range(NCH):
                    ngrp = c + 1
                    ets = {}
                    for g in range(ngrp):
                        for jt in range(4 * g, 4 * g + 4):
                            toff = jt - 4 * c
                            lo = max(toff, 0) * 128
                            w = IC - lo
                            pss = []
                            for hh in range(2):
                                ps_s = sc_ps.tile([128, IC], dt.float32,
                                                  tag="sc")
                                nc.tensor.matmul(
                                    ps_s[:, 0:w],
                                    kT2[hh * HD:hh * HD + HD,
                                        jt * ST:(jt + 1) * ST],
                                    qT2[p][hh * HD:hh * HD + HD,
                                           c * IC + lo:(c + 1) * IC],
                                    start=True, stop=True)
                                pss.append(ps_s)
                                ets[(hh, jt)] = exp_pool.tile(
                                    [128, IC], dt.bfloat16,
                                    name=f"et{hh}{jt % 8}",
                                    tag=f"et{hh}{jt % 8}")
                            for hh in range(2):
                                nc.scalar.activation(
                                    ets[(hh, jt)][:, lo:IC],
                                    pss[hh][:, 0:w], Exp, scale=float(SCALE))
                                if toff >= 0:
                                    nc.vector.tensor_mul(
                                        ets[(hh, jt)][:, lo:lo + 128],
                                        ets[(hh, jt)][:, lo:lo + 128],
                                        dmask[:])
                        for hh in range(2):
                            pavt = av_ps.tile([128, 4 * HD1], dt.float32,
                                              name=f"av{hh}", tag=f"av{hh}")
                            for t in range(4):
                                j0 = 4 * g
                                j1 = min(4 * g + 3, 4 * c + t)
                                for jt in range(j0, j1 + 1):
                                    nc.tensor.matmul(
                                        pavt[:, t * HD1:t * HD1 + HD1],
                                        ets[(hh, jt)][:, t * 128:(t + 1) * 128],
                                        v_aug[:, jt * HD1:(jt + 1) * HD1],
                                        start=(jt == j0), stop=(jt == j1))
                            acc = accs[hh]
                            if g == 0:
                                nc.vector.tensor_copy(acc[:], pavt[:])
                            else:
                                nc.vector.tensor_add(acc[:], acc[:], pavt[:])
                    ys4 = ystage_pool.tile([128, IC], dt.bfloat16, tag="ys")
                    for t in range(4):
                        it = 4 * c + t
                        ypair = ytmp_pool.tile([128, 128], dt.bfloat16,
                                               tag="yp")
                        for hh in range(2):
                            base = t * HD1
                            rec = ytmp_pool.tile([128, 1], dt.float32,
                                                 tag="rec")
                            nc.vector.reciprocal(
                                rec[:], accs[hh][:, base + HD:base + HD + 1])
                            nc.vector.tensor_scalar_mul(
                                ypair[:, hh * HD:(hh + 1) * HD],
                                accs[hh][:, base:base + HD], rec[:])
                        pt = yt_ps.tile([128, 128], dt.bfloat16, tag="yt")
                        nc.tensor.transpose(pt[:], ypair[:], ident[:])
                        nc.vector.tensor_copy(ys4[:, t * 128:(t + 1) * 128],
                                              pt[:])
                    eng = nc.sync if c % 2 == 0 else nc.scalar
                    for j in range(2):
                        eng.dma_start(
                            out=a2a_in[p][2 * c + j, :, :],
                            in_=ys4[:, j * SROWS:(j + 1) * SROWS])
                # per-pair AllToAll as soon as this pair's tiles are staged
                nc.gpsimd.collective_compute(
                    "AllToAll", mybir.AluOpType.bypass,
                    replica_groups=[list(range(N_CORES))],
                    ins=[a2a_in[p][:]], outs=[a2a_out[p][:]])

        # ---- phase 4: output projection (even m-tiles overlap A2A#2) ----
        with (
            tc.tile_pool(name="ytf", bufs=1) as ytf_pool,
            tc.tile_pool(name="o_ps", bufs=1, space="PSUM") as o_ps,
            tc.tile_pool(name="o_sb", bufs=3) as o_sb,
        ):
            ytf = {}
            ps_os = {}
            for p in range(2):
                flat = a2a_out[p][:].rearrange("r m s -> (r m) s")
                for r in range(N_CORES):
                    mt = 2 * r + p
                    ytf[mt] = ytf_pool.tile([128, SROWS], dt.bfloat16,
                                            name=f"ytf{mt}", tag=f"ytf{mt}")
                    eng = nc.sync if r % 2 == 0 else nc.scalar
                    eng.dma_start(out=ytf[mt][:],
                                  in_=flat[r * 128:(r + 1) * 128, :])
                for st in range(2):
                    for nch in range(4):
                        if p == 0:
                            ps_os[(st, nch)] = o_ps.tile(
                                [128, 512], dt.float32,
                                name=f"o{st}{nch}", tag=f"o{st}{nch}")
                        ps_o = ps_os[(st, nch)]
                        for r in range(N_CORES):
                            mt = 2 * r + p
                            m = mt
                            nc.tensor.matmul(
                                ps_o[:], ytf[mt][:, st * 128:(st + 1) * 128],
                                wo_sb[:, m * D + nch * 512:
                                      m * D + (nch + 1) * 512],
                                start=(p == 0 and r == 0),
                                stop=(p == 1 and r == N_CORES - 1))
                        if p == 1:
                            ob = o_sb.tile([128, 512], dt.float32, tag="ob")
                            nc.scalar.copy(ob[:], ps_o[:])
                            eng = nc.sync if nch % 2 == 0 else nc.scalar
                            eng.dma_start(
                                out=out[st * 128:(st + 1) * 128,
                                        nch * 512:(nch + 1) * 512],
                                in_=ob[:])

        wo_pool.release()
        pers.release()
        const.release()

    nc.compile()
    return nc


def _numpy_reference(x, freqs_cos, freqs_sin, input_pos, wq, wk, wv, wo,
                     k_cache, v_cache):
    B, S_, _ = x.shape
    NKV = 8
    n_rep = NH // NKV

    def rope(t, cos, sin):
        tr = t[..., 0::2]
        ti = t[..., 1::2]
        c = cos[None, :, None, :]
        s = sin[None, :, None, :]
        o = np.stack([tr * c - ti * s, tr * s + ti * c], axis=-1)
        return o.reshape(t.shape)

    q = (x @ wq.T).reshape(B, S_, NH, HD)
    k = (x @ wk.T).reshape(B, S_, NKV, HD)
    v = (x @ wv.T).reshape(B, S_, NKV, HD)
    q = rope(q, freqs_cos, freqs_sin).transpose(0, 2, 1, 3)
    k = rope(k, freqs_cos, freqs_sin).transpose(0, 2, 1, 3)
    v = v.transpose(0, 2, 1, 3)
    k_full = np.array(k_cache)
    v_full = np.array(v_cache)
    k_full[:, :, input_pos] = k
    v_full[:, :, input_pos] = v
    mask = np.tril(np.ones((k_full.shape[2], k_full.shape[2]), bool))[input_pos]
    k_rep = np.repeat(k_full, n_rep, axis=1)
    v_rep = np.repeat(v_full, n_rep, axis=1)
    sc = np.einsum("bhsd,bhtd->bhst", q, k_rep) * SCALE
    sc = np.where(mask[None, None], sc, -np.inf)
    sc = sc - sc.max(axis=-1, keepdims=True)
    e = np.exp(sc)
    attn = e / e.sum(axis=-1, keepdims=True)
    y = np.einsum("bhst,bhtd->bhsd", attn, v_rep)
    y = y.transpose(0, 2, 1, 3).reshape(B, S_, NH * HD)
    return (y @ wo.T).astype(np.float32)


def _pair_expand(a, sign_odd=False):
    S_, n = a.shape
    o = np.empty((S_, 2 * n), np.float32)
    o[:, 0::2] = -a if sign_odd else a
    o[:, 1::2] = a
    return o


def _fold_stiles(a):
    W = a.shape[1]
    return np.ascontiguousarray(
        a.reshape(NS, 128, W).transpose(1, 0, 2).reshape(128, NS * W))


def kernel(x, freqs_cos, freqs_sin, input_pos, wq, wk, wv, wo,
           k_cache, v_cache):
    ipos = np.asarray(input_pos)
    if not np.array_equal(ipos, np.arange(S, dtype=ipos.dtype)):
        return _numpy_reference(np.asarray(x, np.float32),
                                np.asarray(freqs_cos), np.asarray(freqs_sin),
                                ipos, np.asarray(wq), np.asarray(wk),
                                np.asarray(wv), np.asarray(wo),
                                np.asarray(k_cache), np.asarray(v_cache))

    from concourse.bass_utils import run_bass_kernel_spmd

    if "nc" not in _CACHE:
        _CACHE["nc"] = _build()
    nc = _CACHE["nc"]

    bf16 = ml_dtypes.bfloat16
    x2 = np.asarray(x, np.float32)[0].astype(bf16)
    xstr = np.ascontiguousarray(x2.reshape(S, ND, DT).transpose(1, 0, 2))
    cos = np.asarray(freqs_cos, np.float32)
    sin = np.asarray(freqs_sin, np.float32)
    cos2 = _fold_stiles(np.tile(_pair_expand(cos), (1, 5)))
    sin2 = _fold_stiles(np.tile(_pair_expand(sin, sign_odd=True), (1, 5)))
    woTf = np.asarray(wo, np.float32).T.astype(bf16)
    woT = np.ascontiguousarray(
        woTf.reshape(ND, 128, D).transpose(1, 0, 2).reshape(128, ND * D))

    in_maps = []
    for c in range(N_CORES):
        wq_c = np.asarray(wq, np.float32)[c * QW:(c + 1) * QW].T
        wk_c = np.asarray(wk, np.float32)[c * HD:(c + 1) * HD].T
        wv_c = np.asarray(wv, np.float32)[c * HD:(c + 1) * HD].T
        wqkvT_f = np.concatenate([wq_c, wk_c, wv_c], axis=1).astype(bf16)
        wqkvT = np.ascontiguousarray(
            wqkvT_f.reshape(ND, 128, QKV).transpose(1, 0, 2)
            .reshape(128, ND * QKV))
        in_maps.append({
            "xstr": xstr, "wqkvT": wqkvT, "woT": woT,
            "cos2": cos2, "sin2": sin2,
        })

    res = run_bass_kernel_spmd(nc, in_maps, core_ids=list(range(N_CORES)),
                               trace=bool(os.environ.get("KERNEL_TRACE")))
    _CACHE["last_res"] = res
    rows = [res.results[c]["out"] for c in range(N_CORES)]
    return np.concatenate(rows, axis=0)[None].astype(np.float32)


# revision 15
# speedup vs baseline: 1.2986x; 1.0078x over previous
"""Tensor-parallel MHA prefill kernel for 8 TRN2 NeuronCores.

Sharding: heads across cores (4 Q heads + 1 KV head per core).
Per core: QKV projection (bf16 matmuls, fp32 accum), interleaved RoPE,
causal attention in scores-transposed orientation (softmax denominators
via an appended ones-column in the AV matmul, accumulated over 4-j-tile
subgroups into SBUF), pair-split AllToAll overlapped with the split
output projection for this core's 256 sequence rows. Host only
slices/transposes/casts weights and the input, and concatenates the 8
output row-blocks.
"""
import os
import numpy as np
import ml_dtypes

N_CORES = 8
S = 2048
D = 2048
NH = 32
HD = 64
HPC = NH // N_CORES      # 4 q heads per core
QW = HPC * HD            # 256
QK = QW + HD             # 320
QKV = QW + 2 * HD        # 384
SCALE = 1.0 / np.sqrt(HD)

ST = 128
NS = S // ST             # 16
DT = 128
ND = D // DT             # 16
NXB = 8                  # stripes transposed via xbar DMA (rest via PE)
IC = 512
NCH = S // IC            # 4
SROWS = S // N_CORES     # 256
HD1 = HD + 1             # 65

_CACHE = {}


def _build():
    from concourse import bacc
    import concourse.mybir as mybir
    from concourse.tile import TileContext
    from concourse.masks import make_identity

    dt = mybir.dt
    Exp = mybir.ActivationFunctionType.Exp
    nc = bacc.Bacc("TRN2", target_bir_lowering=False, debug=False,
                   num_devices=N_CORES)

    xstr = nc.declare_dram_parameter("xstr", [ND, S, DT], dt.bfloat16,
                                     isOutput=False)
    wqkvT = nc.declare_dram_parameter("wqkvT", [128, ND * QKV], dt.bfloat16,
                                      isOutput=False)
    woT = nc.declare_dram_parameter("woT", [128, ND * D], dt.bfloat16,
                                    isOutput=False)
    cos2 = nc.declare_dram_parameter("cos2", [128, NS * QK], dt.float32,
                                     isOutput=False)
    sin2 = nc.declare_dram_parameter("sin2", [128, NS * QK], dt.float32,
                                     isOutput=False)
    out = nc.declare_dram_parameter("out", [SROWS, D], dt.float32, isOutput=True)

    a2a_in = [nc.dram_tensor(f"a2a_in{p}", [N_CORES, 128, SROWS], dt.bfloat16)
              for p in range(2)]
    a2a_out = [nc.dram_tensor(f"a2a_out{p}", [N_CORES, 128, SROWS], dt.bfloat16)
               for p in range(2)]

    with TileContext(nc) as tc:
        const = tc.alloc_tile_pool(name="const", bufs=1)
        ident = const.tile([128, 128], dt.bfloat16, tag="ident")
        make_identity(nc, ident)
        dmask = const.tile([128, 128], dt.bfloat16, tag="dmask")
        nc.gpsimd.memset(dmask[:], 1.0)
        nc.gpsimd.affine_select(
            out=dmask[:], in_=dmask[:], compare_op=mybir.AluOpType.is_ge,
            fill=0.0, base=0, pattern=[[1, 128]], channel_multiplier=-1)

        pers = tc.alloc_tile_pool(name="pers", bufs=1)
        wq_sb = pers.tile([128, ND * QKV], dt.bfloat16, tag="wq")
        cs_sb = pers.tile([128, NS * QK], dt.float32, tag="cs")
        sn_sb = pers.tile([128, NS * QK], dt.float32, tag="sn")
        qT2 = [pers.tile([128, S], dt.bfloat16, name=f"qT{p}", tag=f"qT{p}")
               for p in range(2)]
        kT2 = pers.tile([128, S], dt.bfloat16, tag="kT2")
        v_aug = pers.tile([128, NS * HD1], dt.bfloat16, tag="vaug")
        nc.gpsimd.memset(v_aug[:], 1.0)

        # ---- phase 1+2: x transposes (xbar on sync for stripes < NXB,
        #      PE for the rest), then QKV + RoPE + q/k transposes ----
        with (
            tc.tile_pool(name="xt", bufs=1) as xt_pool,
            tc.tile_pool(name="xn", bufs=4) as xn_pool,
        ):
            xT = [xt_pool.tile([128, S], dt.bfloat16, name=f"xT{i}",
                               tag=f"xT{i}") for i in range(ND)]
            nc.gpsimd.dma_start(out=wq_sb[:], in_=wqkvT[:])
            for i in range(NXB):
                nc.sync.dma_start(out=xT[i][:], in_=xstr[i], transpose=True)
            nc.gpsimd.dma_start(out=cs_sb[:], in_=cos2[:])
            nc.gpsimd.dma_start(out=sn_sb[:], in_=sin2[:])

            with (
                tc.tile_pool(name="qkv_ps", bufs=5, space="PSUM") as qkv_ps,
                tc.tile_pool(name="tr_ps", bufs=3, space="PSUM") as tr_ps,
                tc.tile_pool(name="rope", bufs=3) as rope_pool,
                tc.tile_pool(name="qkrot", bufs=1) as qkrot_pool,
            ):
                # PE transposes for stripes NXB..15, fed by native x rows
                for s in range(NS):
                    xn = xn_pool.tile([128, (ND - NXB) * DT], dt.bfloat16,
                                      tag="xn")
                    nc.gpsimd.dma_start(
                        out=xn[:],
                        in_=xstr[NXB:ND, s * ST:(s + 1) * ST, :]
                        .rearrange("i s d -> s i d"))
                    for i in range(NXB, ND):
                        pt = tr_ps.tile([128, 128], dt.bfloat16, tag="tr")
                        nc.tensor.transpose(
                            pt[:], xn[:, (i - NXB) * DT:(i - NXB + 1) * DT],
                            ident[:])
                        nc.vector.tensor_copy(xT[i][:, s * ST:(s + 1) * ST],
                                              pt[:])
                qkrot = [qkrot_pool.tile([128, QK], dt.bfloat16,
                                         name=f"qk{s}", tag=f"qk{s}")
                         for s in range(NS)]
                for s in range(NS):
                    ps = qkv_ps.tile([128, QKV], dt.float32, tag="qkv")
                    for i in range(ND):
                        nc.tensor.matmul(ps[:], xT[i][:, s * ST:(s + 1) * ST],
                                         wq_sb[:, i * QKV:(i + 1) * QKV],
                                         start=(i == 0), stop=(i == ND - 1))
                    qk_w = rope_pool.tile([128, QK], dt.float32, tag="qkw")
                    nc.scalar.copy(qk_w[:, 0:QK:2], ps[:, 1:QK:2])
                    nc.scalar.copy(qk_w[:, 1:QK:2], ps[:, 0:QK:2])
                    nc.scalar.copy(v_aug[:, s * HD1:s * HD1 + HD],
                                   ps[:, QK:QKV])
                    t1 = rope_pool.tile([128, QK], dt.float32, tag="t1")
                    t2 = rope_pool.tile([128, QK], dt.float32, tag="t2")
                    nc.vector.tensor_mul(t1[:], ps[:, 0:QK],
                                         cs_sb[:, s * QK:(s + 1) * QK])
                    nc.vector.tensor_mul(t2[:], qk_w[:],
                                         sn_sb[:, s * QK:(s + 1) * QK])
                    nc.vector.tensor_add(qkrot[s][:], t1[:], t2[:])
                    for p in range(2):
                        pt = tr_ps.tile([128, 128], dt.bfloat16, tag="tr")
                        nc.tensor.transpose(
                            pt[:], qkrot[s][:, p * 128:(p + 1) * 128], ident[:])
                        nc.scalar.copy(qT2[p][:, s * ST:(s + 1) * ST], pt[:])
                    pt = tr_ps.tile([128, 128], dt.bfloat16, tag="tr")
                    nc.tensor.transpose(pt[0:HD, :], qkrot[s][:, QW:QK],
                                        ident[:])
                    nc.scalar.copy(kT2[0:HD, s * ST:(s + 1) * ST], pt[0:HD, :])
                    nc.scalar.copy(kT2[HD:128, s * ST:(s + 1) * ST],
                                   pt[0:HD, :])

        # ---- preload woT (one 8MB DMA) while attention runs ----
        wo_pool = tc.alloc_tile_pool(name="wo_sb", bufs=1)
        wo_sb = wo_pool.tile([128, ND * D], dt.bfloat16, tag="wo")
        nc.gpsimd.dma_start(out=wo_sb[:], in_=woT[:])

        # ---- phase 3: attention (pair-major; AV in 4-jt subgroups) ----
        with (
            tc.tile_pool(name="sc_ps", bufs=3, space="PSUM") as sc_ps,
            tc.tile_pool(name="av_ps", bufs=1, space="PSUM") as av_ps,
            tc.tile_pool(name="yt_ps", bufs=1, space="PSUM") as yt_ps,
            tc.tile_pool(name="expT", bufs=2) as exp_pool,
            tc.tile_pool(name="acc", bufs=1) as acc_pool,
            tc.tile_pool(name="ytmp", bufs=2) as ytmp_pool,
            tc.tile_pool(name="ystage", bufs=2) as ystage_pool,
        ):
            for p in range(2):
                accs = {}
                for hh in range(2):
                    accs[hh] = acc_pool.tile([128, 4 * HD1], dt.float32,
                                             name=f"acc{hh}", tag=f"acc{hh}")
                for c in range(NCH):
                    ngrp = c + 1
                    ets = {}
                    for g in range(ngrp):
                        for jt in range(4 * g, 4 * g + 4):
                            toff = jt - 4 * c
                            lo = max(toff, 0) * 128
                            w = IC - lo
                            pss = []
                            for hh in range(2):
                                ps_s = sc_ps.tile([128, IC], dt.float32,
                                                  tag="sc")
                                nc.tensor.matmul(
                                    ps_s[:, 0:w],
                                    kT2[hh * HD:hh * HD + HD,
                                        jt * ST:(jt + 1) * ST],
                                    qT2[p][hh * HD:hh * HD + HD,
                                           c * IC + lo:(c + 1) * IC],
                                    start=True, stop=True)
                                pss.append(ps_s)
                                ets[(hh, jt)] = exp_pool.tile(
                                    [128, IC], dt.bfloat16,
                                    name=f"et{hh}{jt % 8}",
                                    tag=f"et{hh}{jt % 8}")
                            for hh in range(2):
                                nc.scalar.activation(
                                    ets[(hh, jt)][:, lo:IC],
                                    pss[hh][:, 0:w], Exp, scale=float(SCALE))
                                if toff >= 0:
                                    nc.vector.tensor_mul(
                                        ets[(hh, jt)][:, lo:lo + 128],
                                        ets[(hh, jt)][:, lo:lo + 128],
                                        dmask[:])
                        for hh in range(2):
                            pavt = av_ps.tile([128, 4 * HD1], dt.float32,
                                              name=f"av{hh}", tag=f"av{hh}")
                            for t in range(4):
                                j0 = 4 * g
                                j1 = min(4 * g + 3, 4 * c + t)
                                for jt in range(j0, j1 + 1):
                                    nc.tensor.matmul(
                                        pavt[:, t * HD1:t * HD1 + HD1],
                                        ets[(hh, jt)][:, t * 128:(t + 1) * 128],
                                        v_aug[:, jt * HD1:(jt + 1) * HD1],
                                        start=(jt == j0), stop=(jt == j1))
                            acc = accs[hh]
                            if g == 0:
                                nc.vector.tensor_copy(acc[:], pavt[:])
                            else:
                                nc.vector.tensor_add(acc[:], acc[:], pavt[:])
                    ys4 = ystage_pool.tile([128, IC], dt.bfloat16, tag="ys")
                    for t in range(4):
                        it = 4 * c + t
                        ypair = ytmp_pool.tile([128, 128], dt.bfloat16,
                                               tag="yp")
                        for hh in range(2):
                            base = t * HD1
                            rec = ytmp_pool.tile([128, 1], dt.float32,
                                                 tag="rec")
                            nc.vector.reciprocal(
                                rec[:], accs[hh][:, base + HD:base + HD + 1])
                            nc.vector.tensor_scalar_mul(
                                ypair[:, hh * HD:(hh + 1) * HD],
                                accs[hh][:, base:base + HD], rec[:])
                        pt = yt_ps.tile([128, 128], dt.bfloat16, tag="yt")
                        nc.tensor.transpose(pt[:], ypair[:], ident[:])
                        nc.vector.tensor_copy(ys4[:, t * 128:(t + 1) * 128],
                                              pt[:])
                    eng = nc.sync if c % 2 == 0 else nc.scalar
                    for j in range(2):
                        eng.dma_start(
                            out=a2a_in[p][2 * c + j, :, :],
                            in_=ys4[:, j * SROWS:(j + 1) * SROWS])
                # per-pair AllToAll as soon as this pair's tiles are staged
                nc.gpsimd.collective_compute(
                    "AllToAll", mybir.AluOpType.bypass,
                    replica_groups=[list(range(N_CORES))],
                    ins=[a2a_in[p][:]], outs=[a2a_out[p][:]])

        # ---- phase 4: output projection (even m-tiles overlap A2A#2) ----
        with (
            tc.tile_pool(name="ytf", bufs=1) as ytf_pool,
            tc.tile_pool(name="o_ps", bufs=1, space="PSUM") as o_ps,
            tc.tile_pool(name="o_sb", bufs=3) as o_sb,
        ):
            ytf = {}
            ps_os = {}
            for p in range(2):
                flat = a2a_out[p][:].rearrange("r m s -> (r m) s")
                for r in range(N_CORES):
                    mt = 2 * r + p
                    ytf[mt] = ytf_pool.tile([128, SROWS], dt.bfloat16,
                                            name=f"ytf{mt}", tag=f"ytf{mt}")
                    eng = nc.sync if r % 2 == 0 else nc.scalar
                    eng.dma_start(out=ytf[mt][:],
                                  in_=flat[r * 128:(r + 1) * 128, :])
                for st in range(2):
                    for nch in range(4):
                        if p == 0:
                            ps_os[(st, nch)] = o_ps.tile(
                                [128, 512], dt.float32,
                                name=f"o{st}{nch}", tag=f"o{st}{nch}")
                        ps_o = ps_os[(st, nch)]
                        for r in range(N_CORES):
                            mt = 2 * r + p
                            m = mt
                            nc.tensor.matmul(
                                ps_o[:], ytf[mt][:, st * 128:(st + 1) * 128],
                                wo_sb[:, m * D + nch * 512:
                                      m * D + (nch + 1) * 512],
                                start=(p == 0 and r == 0),
                                stop=(p == 1 and r == N_CORES - 1))
                        if p == 1:
                            ob = o_sb.tile([128, 512], dt.float32, tag="ob")
                            nc.scalar.copy(ob[:], ps_o[:])
                            eng = nc.sync if nch % 2 == 0 else nc.scalar
                            eng.dma_start(
                                out=out[st * 128:(st + 1) * 128,
                                        nch * 512:(nch + 1) * 512],
                                in_=ob[:])

        wo_pool.release()
        pers.release()
        const.release()

    nc.compile()
    return nc


def _numpy_reference(x, freqs_cos, freqs_sin, input_pos, wq, wk, wv, wo,
                     k_cache, v_cache):
    B, S_, _ = x.shape
    NKV = 8
    n_rep = NH // NKV

    def rope(t, cos, sin):
        tr = t[..., 0::2]
        ti = t[..., 1::2]
        c = cos[None, :, None, :]
        s = sin[None, :, None, :]
        o = np.stack([tr * c - ti * s, tr * s + ti * c], axis=-1)
        return o.reshape(t.shape)

    q = (x @ wq.T).reshape(B, S_, NH, HD)
    k = (x @ wk.T).reshape(B, S_, NKV, HD)
    v = (x @ wv.T).reshape(B, S_, NKV, HD)
    q = rope(q, freqs_cos, freqs_sin).transpose(0, 2, 1, 3)
    k = rope(k, freqs_cos, freqs_sin).transpose(0, 2, 1, 3)
    v = v.transpose(0, 2, 1, 3)
    k_full = np.array(k_cache)
    v_full = np.array(v_cache)
    k_full[:, :, input_pos] = k
    v_full[:, :, input_pos] = v
    mask = np.tril(np.ones((k_full.shape[2], k_full.shape[2]), bool))[input_pos]
    k_rep = np.repeat(k_full, n_rep, axis=1)
    v_rep = np.repeat(v_full, n_rep, axis=1)
    sc = np.einsum("bhsd,bhtd->bhst", q, k_rep) * SCALE
    sc = np.where(mask[None, None], sc, -np.inf)
    sc = sc - sc.max(axis=-1, keepdims=True)
    e = np.exp(sc)
    attn = e / e.sum(axis=-1, keepdims=True)
    y = np.einsum("bhst,bhtd->bhsd", attn, v_rep)
    y = y.transpose(0, 2, 1, 3).reshape(B, S_, NH * HD)
    return (y @ wo.T).astype(np.float32)


def _pair_expand(a, sign_odd=False):
    S_, n = a.shape
    o = np.empty((S_, 2 * n), np.float32)
    o[:, 0::2] = -a if sign_odd else a
    o[:, 1::2] = a
    return o


def _fold_stiles(a):
    W = a.shape[1]
    return np.ascontiguousarray(
        a.reshape(NS, 128, W).transpose(1, 0, 2).reshape(128, NS * W))


def kernel(x, freqs_cos, freqs_sin, input_pos, wq, wk, wv, wo,
           k_cache, v_cache):
    ipos = np.asarray(input_pos)
    if not np.array_equal(ipos, np.arange(S, dtype=ipos.dtype)):
        return _numpy_reference(np.asarray(x, np.float32),
                                np.asarray(freqs_cos), np.asarray(freqs_sin),
                                ipos, np.asarray(wq), np.asarray(wk),
                                np.asarray(wv), np.asarray(wo),
                                np.asarray(k_cache), np.asarray(v_cache))

    from concourse.bass_utils import run_bass_kernel_spmd

    if "nc" not in _CACHE:
        _CACHE["nc"] = _build()
    nc = _CACHE["nc"]

    bf16 = ml_dtypes.bfloat16
    x2 = np.asarray(x, np.float32)[0].astype(bf16)
    xstr = np.ascontiguousarray(x2.reshape(S, ND, DT).transpose(1, 0, 2))
    cos = np.asarray(freqs_cos, np.float32)
    sin = np.asarray(freqs_sin, np.float32)
    cos2 = _fold_stiles(np.tile(_pair_expand(cos), (1, 5)))
    sin2 = _fold_stiles(np.tile(_pair_expand(sin, sign_odd=True), (1, 5)))
    woTf = np.asarray(wo, np.float32).T.astype(bf16)
    woT = np.ascontiguousarray(
        woTf.reshape(ND, 128, D).transpose(1, 0, 2).reshape(128, ND * D))

    in_maps = []
    for c in range(N_CORES):
        wq_c = np.asarray(wq, np.float32)[c * QW:(c + 1) * QW].T
        wk_c = np.asarray(wk, np.float32)[c * HD:(c + 1) * HD].T
        wv_c = np.asarray(wv, np.float32)[c * HD:(c + 1) * HD].T
        wqkvT_f = np.concatenate([wq_c, wk_c, wv_c], axis=1).astype(bf16)
        wqkvT = np.ascontiguousarray(
            wqkvT_f.reshape(ND, 128, QKV).transpose(1, 0, 2)
            .reshape(128, ND * QKV))
        in_maps.append({
            "xstr": xstr, "wqkvT": wqkvT, "woT": woT,
            "cos2": cos2, "sin2": sin2,
        })

    res = run_bass_kernel_spmd(nc, in_maps, core_ids=list(range(N_CORES)),
                               trace=bool(os.environ.get("KERNEL_TRACE")))
    _CACHE["last_res"] = res
    rows = [res.results[c]["out"] for c in range(N_CORES)]
    return np.concatenate(rows, axis=0)[None].astype(np.float32)
